# revision 44
# baseline (speedup 1.0000x reference)
"""
Trainium2 Bass kernel for nn_Attention_335007449901 (sparse window attention).

Model (per image, eval mode):
  q = BN(conv1x1(x, wq)); k = BN(conv1x1(x, wk)); v = BN(conv1x1(x, wv))
  7x7 windows over the 112x112 image -> T=256 window tokens, token
  features = (channel, within-window position p) pairs.
  dots[i,j] = <q_i, k_j> * 0.125 ; attn = softmax_j ; out = attn @ v
  y = gelu(out); z = BN(conv1x1(y, wo) + bo); out = gelu(z + x)

Sharding: pure data parallel over batch, 4 images per core on 8 cores.

Implementation notes:
  * BatchNorms folded into conv weights on the host; SCALE folded into q's
    path; k's bias drops (softmax shift invariance along the normalized
    axis); v's bias passes through the attention average (rows sum to 1)
    into the first gelu's bias; the final conv bias + BN fold into the last
    gelu's bias.
  * q and k never materialize: dots_T[j,i] = sum_p x_pj^T M x_pi with
    M = wk_f^T wq_f precomputed on the host, computed as u_p = M^T x_p
    then dots_T += u_p^T x_p. q's bias contributes a per-row term
    c[j] = sum_p (wk_f^T Bq) . x_p[:,j], accumulated with rank-reduce
    matmuls and injected into dots via two rank-1 matmuls.
  * All matmul operands are bf16 (fp32 PSUM accumulation): fp32 matmuls on
    trn2 run as LOW/HIGH double passes, and strided moving operands stream
    ~5x slower, so a window-permuted contiguous bf16 copy of x (x_winb)
    feeds every matmul.
  * dots are computed transposed so softmax normalization is a ones-vector
    matmul reduce; no max subtraction needed (|dots| < ~30, fp32 exp safe).
  * The attention-average is reordered as wv @ (x @ attn^T) ("z-form"):
    the host supplies a pixel-major transposed copy of x, DMA-gathered
    into token-major SBUF blocks (contiguous 1536B runs), so z = x@attn^T
    runs with x^T as the stationary operand - no v-conv, ~25% fewer PE
    rows and half the PSUM->SBUF drain volume of the v-based form.
  * The residual add is a PE matmul accumulation (x^T block stationary,
    identity moving) - no Vector-engine tensor_tensor adds.
  * The final gelu writes CONTIGUOUS window-layout tiles that are DMA'd out
    per group as they complete (a strided in-image scatter write costs ~2x
    on the ACT engine and serializes the store into an end-of-image tail);
    the host un-permutes the window-layout result back to image layout.
  * Cross-image software pipelining: phase 1 of image i+1 (u/dots/c) is
    emitted interleaved with the tail groups of image i's phase 2 so the
    PE stream never drains at image boundaries (keeps the tensor engine's
    DVFS p-state at max).
"""

import numpy as np

IN_C = 128
HIDE_C = 256
HC2 = 128
OUT_C = 128
WS = 7
SCALE = 0.125
EPS = 1e-5
B, H, W = 32, 112, 112
HW = H * W          # 12544
H1 = H // WS        # 16
W1 = W // WS        # 16
T = H1 * W1         # 256 windows
NP = WS * WS        # 49 positions
NCORES = 8
BPC = B // NCORES   # images per core

F32 = np.float32


def _pgroups():
    """Groups of 1-2 within-window positions with a uniform pixel-offset
    stride between members (one strided access pattern per group in the
    image layout). 49 positions -> 24 pairs + 1 singleton. g[2] is the
    group's column base in the position-major window layout x_winb."""
    groups = []
    base = 0
    for ws1 in range(WS):
        for b2 in range(3):
            groups.append(((ws1, 2 * b2), (ws1, 2 * b2 + 1), base))
            base += 2 * T
    for a in range(3):
        groups.append(((2 * a, 6), (2 * a + 1, 6), base))
        base += 2 * T
    groups.append(((6, 6), None, base))
    return groups


def build_bass_kernel(bpc=BPC):
    import concourse.bass as bass
    import concourse.tile as tile
    import concourse.mybir as mybir
    from concourse import bacc

    f32 = mybir.dt.float32
    bf16 = mybir.dt.bfloat16
    AF = mybir.ActivationFunctionType

    nc = bacc.Bacc("TRN2", target_bir_lowering=False)

    # x is pre-converted to bf16 on the host: every on-device consumer
    # (window permute -> x_winb) is bf16 anyway, and this halves both the
    # load DMA volume and the x_img SBUF footprint (enabling bufs=3 so
    # loads are never gated on a recent permute's completion)
    x_d = nc.dram_tensor("x", [bpc, IN_C, HW], bf16, kind="ExternalInput")
    # pixel-major transpose of x for token-major (stationary-side) gathers
    xT_d = nc.dram_tensor("xT", [bpc, HW, IN_C], bf16, kind="ExternalInput")
    # host-permuted window-layout x; only image 0 reads it (loading it
    # directly skips the on-device permute that otherwise paces startup)
    xwin_d = nc.dram_tensor("xwin", [bpc, IN_C, NP * T], bf16,
                            kind="ExternalInput")
    m_d = nc.dram_tensor("m", [IN_C, IN_C], bf16, kind="ExternalInput")
    h_d = nc.dram_tensor("hcol", [IN_C, 1], bf16, kind="ExternalInput")
    wvT_d = nc.dram_tensor("wvT", [IN_C, HIDE_C], bf16, kind="ExternalInput")
    woT_d = nc.dram_tensor("woT", [HIDE_C, OUT_C], bf16, kind="ExternalInput")
    id_d = nc.dram_tensor("ident", [IN_C, IN_C], bf16, kind="ExternalInput")
    # packed per-partition fp32 bias columns: [Bv_lo, Bv_hi, Bo]
    bias_d = nc.dram_tensor("biases", [128, 3], f32, kind="ExternalInput")
    # output stays in window-permuted layout; the host un-permutes
    out_d = nc.dram_tensor("out", [bpc, OUT_C, NP * T], f32,
                           kind="ExternalOutput")

    groups = _pgroups()
    NEARLY = 6
    FRONT = 17   # phase-2 groups emitted before the next image's phase 1

    with tile.TileContext(nc) as tc:
        with (
            tc.tile_pool(name="singles", bufs=1) as singles,
            tc.tile_pool(name="xpool", bufs=3) as xpool,
            tc.tile_pool(name="xwin", bufs=2) as xwin_pool,
            tc.tile_pool(name="u_sb", bufs=4) as u_sb_pool,
            tc.tile_pool(name="xt_sb", bufs=8) as xt_pool,
            tc.tile_pool(name="z_sb", bufs=4) as z_sb_pool,
            tc.tile_pool(name="g_sb", bufs=3) as g_sb_pool,
            tc.tile_pool(name="attn_sb", bufs=2) as attn_pool,
            tc.tile_pool(name="ow_sb", bufs=6) as ow_pool,
            tc.tile_pool(name="small_sb", bufs=2) as small_pool,
            tc.tile_pool(name="ps_work", bufs=2, space="PSUM") as ps_work,
            tc.tile_pool(name="ps_dots", bufs=1, space="PSUM") as ps_dots,
            tc.tile_pool(name="ps_z", bufs=1, space="PSUM") as ps_z_pool,
            tc.tile_pool(name="ps_av", bufs=2, space="PSUM") as ps_av,
            tc.tile_pool(name="ps_o", bufs=2, space="PSUM") as ps_o_pool,
        ):
            # ---- weights / constants (resident) ----
            m_sb = singles.tile([128, IN_C], bf16)
            nc.sync.dma_start(out=m_sb, in_=m_d.ap())
            h_sb = singles.tile([128, 1], bf16)
            nc.sync.dma_start(out=h_sb, in_=h_d.ap())
            wvT = singles.tile([128, HIDE_C], bf16)
            nc.sync.dma_start(out=wvT, in_=wvT_d.ap())
            woT = singles.tile([128, 2, OUT_C], bf16)
            nc.sync.dma_start(
                out=woT, in_=woT_d.ap().rearrange("(kc p) m -> p kc m", kc=2)
            )
            biases = singles.tile([128, 3], f32)
            nc.sync.dma_start(out=biases, in_=bias_d.ap())
            bv_ap = [biases[:, 0:1], biases[:, 1:2]]
            bo_ap = biases[:, 2:3]
            ident = singles.tile([128, IN_C], bf16)
            nc.sync.dma_start(out=ident, in_=id_d.ap())

            ones_mat = singles.tile([128, 128], bf16)
            nc.vector.memset(ones_mat, 1.0)
            ones_row = singles.tile([1, T], bf16)
            nc.vector.memset(ones_row, 1.0)
            scratch = singles.tile([128, 1], f32)
            # preload the EXP ACT table once, off any critical chain
            nc.scalar.activation(scratch, biases[:, 0:1], AF.Exp)

            st = [dict() for _ in range(bpc)]
            chunk_starts = list(range(0, NP, 2))   # 2 positions per chunk

            def emit_load(img):
                x_img = xpool.tile([128, HW], bf16, tag="ximg")
                for dc in range(4):
                    nc.sync.dma_start(
                        out=x_img[:, dc * (HW // 4):(dc + 1) * (HW // 4)],
                        in_=x_d.ap()[img, :, dc * (HW // 4):
                                     (dc + 1) * (HW // 4)])
                st[img]["x5"] = x_img.rearrange(
                    "p (h a w b) -> p h a w b", h=H1, a=WS, b=WS)
                st[img]["xT5"] = xT_d.ap()[img].rearrange(
                    "(h a w b) c -> h a w b c", h=H1, a=WS, b=WS)
                st[img]["xt"] = {}

            def grp_src(img, g):
                """strided image-layout AP of this group's positions"""
                x5 = st[img]["x5"]
                (ws1, ws2), p2, _ = g
                if p2 is None:
                    return x5[:, :, ws1, :, ws2]
                if p2[0] == ws1:  # within-row pair, pixel stride 1
                    return x5[:, :, ws1, :, ws2:ws2 + 2].rearrange(
                        "p h w b -> p b h w")
                return x5[:, :, ws1:ws1 + 2, :, ws2].rearrange(
                    "p h a w -> p a h w")

            def emit_winb_load(img):
                x_winb = xwin_pool.tile([128, NP * T], bf16, tag="xwin")
                st[img]["xw"] = x_winb
                for dc in range(4):
                    nc.sync.dma_start(
                        out=x_winb[:, dc * (NP * T // 4):
                                   (dc + 1) * (NP * T // 4)],
                        in_=xwin_d.ap()[img, :, dc * (NP * T // 4):
                                        (dc + 1) * (NP * T // 4)])

            def emit_permute(img, spread=True):
                """Position-major bf16 window copy, split across the three
                copy engines: GPSIMD alone is ~1.9us/copy and a serial
                25-copy chain gates the next image's load (x_img buffer
                reuse) and, through the Sync DMA sequencer, the stores."""
                x_winb = xwin_pool.tile([128, NP * T], bf16, tag="xwin")
                st[img]["xw"] = x_winb
                for gi, g in enumerate(groups):
                    N = T if g[1] is None else 2 * T
                    dst = x_winb[:, g[2]:g[2] + N]
                    src = grp_src(img, g)
                    if gi % 4 == 2:
                        nc.scalar.activation(dst, src, AF.Copy, scale=1.0)
                    elif gi % 4 == 3:
                        nc.vector.tensor_copy(dst, src)
                    else:
                        nc.gpsimd.tensor_copy(dst, src)

            # ---- phase 1: dots_T and c accumulation over positions ----
            def p1_steps(img):
                xw = st[img]["xw"]
                pend = []

                def u_conv(ci, p0):
                    npos = min(2, NP - p0)
                    N = npos * T
                    base = p0 * T
                    u_ps = ps_work.tile([128, 512], f32, tag="pwork")
                    nc.tensor.matmul(u_ps[:, :N], lhsT=m_sb,
                                     rhs=xw[:, base:base + N],
                                     start=True, stop=True)
                    u_sbt = u_sb_pool.tile([128, 512], bf16, tag="u")
                    if ci % 3 == 0:
                        nc.scalar.activation(u_sbt[:, :N], u_ps[:, :N],
                                             AF.Copy, scale=1.0)
                    else:
                        nc.vector.tensor_copy(u_sbt[:, :N], u_ps[:, :N])
                    return u_sbt

                def dots_mms(ci, p0, u_sbt):
                    dots = st[img]["dots"]
                    npos = min(2, NP - p0)
                    base = p0 * T
                    first = ci == 0
                    for pi in range(npos):
                        for jh in (0, 1):
                            nc.tensor.matmul(
                                dots[jh],
                                lhsT=u_sbt[:, pi * T + jh * 128:
                                           pi * T + jh * 128 + 128],
                                rhs=xw[:, base + pi * T:
                                       base + (pi + 1) * T],
                                start=first and pi == 0 and jh == 0,
                                stop=False,
                                skip_group_check=True)

                def mk_chunk(ci, p0):
                    def f():
                        if ci == 0:
                            dots_t = ps_dots.tile([128, 512], f32,
                                                  tag="dots", name="dots")
                            st[img]["dots"] = [dots_t[:, 0:T],
                                               dots_t[:, T:2 * T]]
                        u_sbt = u_conv(ci, p0)
                        if len(pend) >= 2:
                            dots_mms(*pend.pop(0))
                        pend.append((ci, p0, u_sbt))
                    return f

                def c_step():
                    for pe_ in pend:
                        dots_mms(*pe_)
                    pend.clear()
                    # c[j] = sum_p h . x_p[:, j] (h stays loaded), added
                    # into dots via two rank-1 matmuls
                    dots = st[img]["dots"]
                    c_row_ps = ps_o_pool.tile([1, T], f32, tag="ops",
                                              name="cps")
                    for p in range(NP):
                        nc.tensor.matmul(c_row_ps, lhsT=h_sb,
                                         rhs=xw[:, p * T:(p + 1) * T],
                                         start=p == 0, stop=p == NP - 1)
                    c_row = small_pool.tile([1, T], bf16, tag="csb")
                    nc.vector.tensor_copy(c_row, c_row_ps)
                    for jh in (0, 1):
                        nc.tensor.matmul(
                            dots[jh],
                            lhsT=c_row[:, jh * 128:jh * 128 + 128],
                            rhs=ones_row, start=False, stop=jh == 1,
                            skip_group_check=True)

                steps = [mk_chunk(ci, p0)
                         for ci, p0 in enumerate(chunk_starts)]
                steps.append(c_step)
                return steps

            # ---- token-major x^T gathers (one DMA unit covers a row of
            # up to 3 groups; contiguous (b, c) runs of 1536B) ----
            def emit_gather(img, u):
                xT5 = st[img]["xT5"]
                if u < 7:
                    t = xt_pool.tile([128, 2, 768], bf16, tag="xtrow")
                    for jh in (0, 1):
                        nc.sync.dma_start(
                            out=t[:, jh, :],
                            in_=xT5[jh * 8:jh * 8 + 8, u, :, 0:6, :])
                elif u < 10:
                    a2 = 2 * (u - 7)
                    t = xt_pool.tile([128, 2, 256], bf16, tag="xtcol")
                    for jh in (0, 1):
                        for pi in (0, 1):
                            nc.sync.dma_start(
                                out=t[:, jh, pi * 128:(pi + 1) * 128],
                                in_=xT5[jh * 8:jh * 8 + 8, a2 + pi, :, 6, :])
                else:
                    t = xt_pool.tile([128, 2, 128], bf16, tag="xtsing")
                    for jh in (0, 1):
                        nc.sync.dma_start(
                            out=t[:, jh, :],
                            in_=xT5[jh * 8:jh * 8 + 8, 6, :, 6, :])
                st[img]["xt"][u] = t

            def xt_block(img, gi, pi, jh):
                """[K=128 tokens of half jh, M=128 ch] stationary block for
                position pi of group gi."""
                if gi < 21:
                    t = st[img]["xt"][gi // 3]
                    off = ((gi % 3) * 2 + pi) * 128
                elif gi < 24:
                    t = st[img]["xt"][7 + (gi - 21)]
                    off = pi * 128
                else:
                    t = st[img]["xt"][10]
                    off = 0
                return t[:, jh, off:off + 128]

            def unit_of(gi):
                return gi // 3 if gi < 21 else 7 + min(gi - 21, 3)

            # ---- softmax over j (= partitions of dots_T) ----
            def emit_softmax(img):
                # all x^T gathers for this image up front: they depend
                # only on DRAM, and early issue keeps the 900ns DMA
                # semaphore propagation off every z-matmul's critical path
                for u in range(11):
                    emit_gather(img, u)
                dots = st[img]["dots"]
                attn = [attn_pool.tile([128, T], bf16, tag=f"attn{jc}",
                                       name=f"attn{jc}") for jc in (0, 1)]
                for jc in (0, 1):
                    nc.scalar.activation(attn[jc], dots[jc], AF.Exp)
                s_ps = ps_dots.tile([128, T], f32, tag="dots", name="ssum")
                for jc in (0, 1):
                    nc.tensor.matmul(s_ps, lhsT=ones_mat, rhs=attn[jc],
                                     start=jc == 0, stop=jc == 1)
                r_sb = small_pool.tile([128, T], f32, tag="rsb")
                # sums are positive and O(1..300): the ~18-bit approximate
                # reciprocal is plenty and ~5x faster on the critical chain
                nc.vector.reciprocal_approx_fast(r_sb, s_ps)
                # dummy gelu: pull the GELU ACT-table load off the
                # av(0) -> gelu1(0) -> out(0) critical chain
                nc.scalar.activation(scratch, biases[:, 0:1], AF.Gelu)
                for jc in (0, 1):
                    nc.vector.tensor_mul(attn[jc], attn[jc], r_sb)
                st[img]["attn"] = attn
                st[img]["zcache"] = {}

            # ---- phase 2: z = x @ attn^T, av = wv @ z, out-conv,
            # residual, store ----
            GLOOK = 12  # gather-unit prefetch distance (groups)

            def p2_steps(img):
                attn = st[img]["attn"]
                pend_out = [None]

                def emit_out(gi, g, g_tiles):
                    cnt = 1 if g[1] is None else 2
                    N = cnt * T
                    o_ps = ps_o_pool.tile([128, 512], f32, tag="ops")
                    for pi in range(cnt):
                        for kc in (0, 1):
                            nc.tensor.matmul(
                                o_ps[:, pi * T:(pi + 1) * T],
                                lhsT=woT[:, kc, :],
                                rhs=g_tiles[kc][:, pi * T:(pi + 1) * T],
                                start=kc == 0, stop=False)
                        # residual add on PE: x^T block stationary,
                        # identity moving
                        for jh in (0, 1):
                            nc.tensor.matmul(
                                o_ps[:, pi * T + jh * 128:
                                     pi * T + jh * 128 + 128],
                                lhsT=xt_block(img, gi, pi, jh),
                                rhs=ident,
                                start=False, stop=True,
                                skip_group_check=True)
                    # final gelu into a contiguous window-layout tile,
                    # stored immediately (host un-permutes)
                    ow = ow_pool.tile([128, 512], f32, tag="ow")
                    nc.scalar.activation(ow[:, :N], o_ps[:, :N], AF.Gelu,
                                         bias=bo_ap, scale=1.0)
                    nc.sync.dma_start(
                        out=out_d.ap()[img, :, g[2]:g[2] + N],
                        in_=ow[:, :N])

                zcache = st[img]["zcache"]

                def emit_z(gi, g):
                    """z_p = x_p @ attn^T: x^T blocks stationary; drained
                    to bf16 SBUF one group ahead of the wv matmuls."""
                    cnt = 1 if g[1] is None else 2
                    z_ps = ps_z_pool.tile([128, 512], f32, tag="z")
                    zsb = []
                    for pi in range(cnt):
                        for jh in (0, 1):
                            nc.tensor.matmul(
                                z_ps[:, pi * T:(pi + 1) * T],
                                lhsT=xt_block(img, gi, pi, jh),
                                rhs=attn[jh],
                                start=jh == 0, stop=jh == 1)
                        z_t = z_sb_pool.tile([128, T], bf16, tag="zsb")
                        if gi % 3 == 2:
                            nc.scalar.activation(z_t, z_ps[:, pi * T:
                                                 (pi + 1) * T],
                                                 AF.Copy, scale=1.0)
                        else:
                            nc.vector.tensor_copy(z_t, z_ps[:, pi * T:
                                                  (pi + 1) * T])
                        zsb.append(z_t)
                    zcache[gi] = zsb

                def mk_group(gi, g):
                    def f():
                        cnt = 1 if g[1] is None else 2
                        N = cnt * T
                        if gi == 0:
                            emit_z(0, groups[0])
                        if gi + 1 < len(groups):
                            emit_z(gi + 1, groups[gi + 1])
                        zsb = zcache.pop(gi)
                        g_tiles = []
                        for kc in (0, 1):
                            av = ps_av.tile([128, 512], f32, tag="av",
                                            name=f"av{kc}")
                            for pi in range(cnt):
                                nc.tensor.matmul(
                                    av[:, pi * T:(pi + 1) * T],
                                    lhsT=wvT[:, kc * 128:kc * 128 + 128],
                                    rhs=zsb[pi],
                                    start=True, stop=True)
                            g_t = g_sb_pool.tile([128, 512], bf16,
                                                 tag=f"g{kc}")
                            nc.scalar.activation(g_t[:, :N], av[:, :N],
                                                 AF.Gelu, bias=bv_ap[kc],
                                                 scale=1.0)
                            g_tiles.append(g_t)
                        # out-conv lags one group so PE never waits on gelu1
                        if pend_out[0] is not None:
                            emit_out(*pend_out[0])
                        pend_out[0] = (gi, g, g_tiles)
                    return f

                steps = [mk_group(gi, g) for gi, g in enumerate(groups)]
                steps.append(lambda: emit_out(*pend_out[0]))
                return steps

            # ---- emission schedule: splice phase 1 of image i into the
            # tail of phase 2 of image i-1 ----
            emit_load(0)
            emit_winb_load(0)
            tail = []
            for img in range(bpc):
                p1 = p1_steps(img)
                # interleave previous image's phase-2 tail with this phase 1
                ti = 0
                for si, s in enumerate(p1):
                    s()
                    if (si + 1) % 3 == 0 and ti < len(tail):
                        tail[ti]()
                        ti += 1
                while ti < len(tail):
                    tail[ti]()
                    ti += 1
                if img + 1 < bpc:
                    emit_load(img + 1)
                emit_softmax(img)
                p2 = p2_steps(img)
                for s in p2[:FRONT]:
                    s()
                tail = p2[FRONT:]
                if img + 1 < bpc:
                    emit_permute(img + 1)
            for s in tail:
                s()

    nc.compile()
    return nc


def fold_params(wq, gq, bq, mq, vq, wk, gk, bk, mk, vk,
                wv, gv, bv, mv, vv, wo, bo, go, bbo, mo, vo):
    """Host-side BN/bias folding. Returns (M, h, wvT, woT, biases, ident)."""
    import ml_dtypes
    bf16 = ml_dtypes.bfloat16

    aq = gq / np.sqrt(vq + EPS)
    wq_f = (SCALE * aq)[:, None] * wq
    Bq = SCALE * (bq - aq * mq)

    ak = gk / np.sqrt(vk + EPS)
    wk_f = ak[:, None] * wk          # k bias drops (softmax shift invariance)

    M = wk_f.T @ wq_f                # dots_T = sum_p (M^T x_p)^T x_p
    hv = wk_f.T @ Bq                 # c[j] = sum_p hv . x_p[:, j]

    av = gv / np.sqrt(vv + EPS)
    wv_f = av[:, None] * wv
    Bv = bv - av * mv                # applied inside the first gelu

    ao = go / np.sqrt(vo + EPS)
    wo_f = ao[:, None] * wo
    Bo = ao * (bo - mo) + bbo        # conv bias + BN fold, inside last gelu

    biases = np.stack([Bv[:128], Bv[128:], Bo], axis=1).astype(F32)
    return (np.ascontiguousarray(M).astype(bf16),
            np.ascontiguousarray(hv[:, None]).astype(bf16),
            np.ascontiguousarray(wv_f.T).astype(bf16),
            np.ascontiguousarray(wo_f.T).astype(bf16),
            biases,
            np.eye(IN_C, dtype=bf16))


_CACHED = {}


def _get_nc(bpc=BPC):
    if bpc not in _CACHED:
        _CACHED[bpc] = build_bass_kernel(bpc)
    return _CACHED[bpc]


def make_in_maps(inputs):
    x = np.asarray(inputs["x"], F32)
    m, hv, wvT, woT, biases, ident = fold_params(
        *[np.asarray(inputs[k], F32) for k in
          ("wq", "gq", "bq", "mq", "vq", "wk", "gk", "bk", "mk", "vk",
           "wv", "gv", "bv", "mv", "vv", "wo", "bo", "go", "bbo", "mo", "vo")]
    )
    import ml_dtypes
    xb = x.astype(ml_dtypes.bfloat16)
    in_maps = []
    for c in range(NCORES):
        xs = np.ascontiguousarray(
            xb[c * BPC:(c + 1) * BPC].reshape(BPC, IN_C, HW))
        xt = np.ascontiguousarray(xs.transpose(0, 2, 1))
        xw = np.ascontiguousarray(_permute_host(
            xs.reshape(BPC, IN_C, H, W)))
        in_maps.append({"x": xs, "xT": xt, "xwin": xw, "m": m, "hcol": hv,
                        "wvT": wvT, "woT": woT, "biases": biases,
                        "ident": ident})
    return in_maps


def _blk_map():
    """Device window-layout block index for window position (a, b)."""
    blk = np.empty((WS, WS), np.int64)
    for a in range(WS):
        for b in range(WS):
            if b < 6:
                blk[a, b] = (3 * a + b // 2) * 2 + (b % 2)
            elif a < 6:
                blk[a, b] = 42 + (a // 2) * 2 + (a % 2)
            else:
                blk[a, b] = 48
    return blk


def _permute_host(x):
    """[N, C, H, W] image layout -> [N, C, NP*T] device window layout."""
    blk = _blk_map()
    order = np.empty(NP, np.int64)
    for a in range(WS):
        for b in range(WS):
            order[blk[a, b]] = a * WS + b
    t = x.reshape(x.shape[0], IN_C, H1, WS, W1, WS)
    t = t.transpose(0, 1, 3, 5, 2, 4)         # n c a b h w
    t = t.reshape(x.shape[0], IN_C, NP, T)[:, :, order]
    return np.ascontiguousarray(t.reshape(x.shape[0], IN_C, NP * T))


def _unpermute_host(res):
    """[BPC, C, NP*T] window-layout -> [BPC, C, H, W] image layout."""
    blk = _blk_map()
    t = res.reshape(res.shape[0], OUT_C, NP, H1, W1)[:, :, blk.ravel()]
    t = t.reshape(res.shape[0], OUT_C, WS, WS, H1, W1)      # c a b h w
    t = t.transpose(0, 1, 4, 2, 5, 3)                       # c h a w b
    return np.ascontiguousarray(
        t.reshape(res.shape[0], OUT_C, H, W))


def kernel(**inputs):
    from concourse.bass_utils import run_bass_kernel_spmd

    in_maps = make_in_maps(inputs)
    nc = _get_nc(BPC)
    res = run_bass_kernel_spmd(nc, in_maps, list(range(NCORES)))
    outs = [_unpermute_host(res.results[c]["out"].reshape(BPC, OUT_C, NP * T))
            for c in range(NCORES)]
    return np.concatenate(outs, axis=0)


# revision 45
# speedup vs baseline: 1.2155x; 1.2155x over previous
"""
Trainium2 Bass kernel for nn_Attention_335007449901 (sparse window attention).

Model (per image, eval mode):
  q = BN(conv1x1(x, wq)); k = BN(conv1x1(x, wk)); v = BN(conv1x1(x, wv))
  7x7 windows over the 112x112 image -> T=256 window tokens, token
  features = (channel, within-window position p) pairs.
  dots[i,j] = <q_i, k_j> * 0.125 ; attn = softmax_j ; out = attn @ v
  y = gelu(out); z = BN(conv1x1(y, wo) + bo); out = gelu(z + x)

Sharding: pure data parallel over batch, 4 images per core on 8 cores.

Implementation notes:
  * BatchNorms folded into conv weights on the host; SCALE folded into q's
    path; k's bias drops (softmax shift invariance along the normalized
    axis); v's bias passes through the attention average (rows sum to 1)
    into the first gelu's bias; the final conv bias + BN fold into the last
    gelu's bias.
  * q and k never materialize: dots_T[j,i] = sum_p x_pj^T M x_pi with
    M = wk_f^T wq_f precomputed on the host, computed as u_p = M^T x_p
    then dots_T += u_p^T x_p. q's bias contributes a per-row term
    c[j] = sum_p (wk_f^T Bq) . x_p[:,j], accumulated with rank-reduce
    matmuls and injected into dots via two rank-1 matmuls.
  * All matmul operands are bf16 (fp32 PSUM accumulation): fp32 matmuls on
    trn2 run as LOW/HIGH double passes, and strided moving operands stream
    ~5x slower, so a window-permuted contiguous bf16 copy of x (x_winb)
    feeds every matmul.
  * dots are computed transposed so softmax normalization is a ones-vector
    matmul reduce; no max subtraction needed (|dots| < ~30, fp32 exp safe).
  * The attention-average is reordered as wv @ (x @ attn^T) ("z-form"):
    the host supplies a pixel-major transposed copy of x, DMA-gathered
    into token-major SBUF blocks (contiguous 1536B runs), so z = x@attn^T
    runs with x^T as the stationary operand - no v-conv, ~25% fewer PE
    rows and half the PSUM->SBUF drain volume of the v-based form.
  * The residual add is a PE matmul accumulation (x^T block stationary,
    identity moving) - no Vector-engine tensor_tensor adds.
  * The final gelu writes CONTIGUOUS window-layout tiles that are DMA'd out
    per group as they complete (a strided in-image scatter write costs ~2x
    on the ACT engine and serializes the store into an end-of-image tail);
    the host un-permutes the window-layout result back to image layout.
  * Cross-image software pipelining: phase 1 of image i+1 (u/dots/c) is
    emitted interleaved with the tail groups of image i's phase 2 so the
    PE stream never drains at image boundaries (keeps the tensor engine's
    DVFS p-state at max).
"""

import numpy as np

IN_C = 128
HIDE_C = 256
HC2 = 128
OUT_C = 128
WS = 7
SCALE = 0.125
EPS = 1e-5
B, H, W = 32, 112, 112
HW = H * W          # 12544
H1 = H // WS        # 16
W1 = W // WS        # 16
T = H1 * W1         # 256 windows
NP = WS * WS        # 49 positions
NCORES = 8
BPC = B // NCORES   # images per core

F32 = np.float32


def _pgroups():
    """Groups of 1-2 within-window positions with a uniform pixel-offset
    stride between members (one strided access pattern per group in the
    image layout). 49 positions -> 24 pairs + 1 singleton. g[2] is the
    group's column base in the position-major window layout x_winb."""
    groups = []
    base = 0
    for ws1 in range(WS):
        for b2 in range(3):
            groups.append(((ws1, 2 * b2), (ws1, 2 * b2 + 1), base))
            base += 2 * T
    for a in range(3):
        groups.append(((2 * a, 6), (2 * a + 1, 6), base))
        base += 2 * T
    groups.append(((6, 6), None, base))
    return groups


def build_bass_kernel(bpc=BPC):
    import concourse.bass as bass
    import concourse.tile as tile
    import concourse.mybir as mybir
    from concourse import bacc

    f32 = mybir.dt.float32
    bf16 = mybir.dt.bfloat16
    AF = mybir.ActivationFunctionType

    nc = bacc.Bacc("TRN2", target_bir_lowering=False)

    # x is pre-converted to bf16 on the host: every on-device consumer
    # (window permute -> x_winb) is bf16 anyway, and this halves both the
    # load DMA volume and the x_img SBUF footprint (enabling bufs=3 so
    # loads are never gated on a recent permute's completion)
    x_d = nc.dram_tensor("x", [bpc, IN_C, HW], bf16, kind="ExternalInput")
    # pixel-major transpose of x for token-major (stationary-side) gathers
    xT_d = nc.dram_tensor("xT", [bpc, HW, IN_C], bf16, kind="ExternalInput")
    # host-permuted window-layout x; only image 0 reads it (loading it
    # directly skips the on-device permute that otherwise paces startup)
    xwin_d = nc.dram_tensor("xwin", [bpc, IN_C, NP * T], bf16,
                            kind="ExternalInput")
    m_d = nc.dram_tensor("m", [IN_C, IN_C], bf16, kind="ExternalInput")
    h_d = nc.dram_tensor("hcol", [IN_C, 1], bf16, kind="ExternalInput")
    wvT_d = nc.dram_tensor("wvT", [IN_C, HIDE_C], bf16, kind="ExternalInput")
    woT_d = nc.dram_tensor("woT", [HIDE_C, OUT_C], bf16, kind="ExternalInput")
    id_d = nc.dram_tensor("ident", [IN_C, IN_C], bf16, kind="ExternalInput")
    # packed per-partition fp32 bias columns: [Bv_lo, Bv_hi, Bo]
    bias_d = nc.dram_tensor("biases", [128, 3], f32, kind="ExternalInput")
    # output stays in window-permuted layout; the host un-permutes
    out_d = nc.dram_tensor("out", [bpc, OUT_C, NP * T], f32,
                           kind="ExternalOutput")

    groups = _pgroups()
    NEARLY = 6
    FRONT = 17   # phase-2 groups emitted before the next image's phase 1

    with tile.TileContext(nc) as tc:
        with (
            tc.tile_pool(name="singles", bufs=1) as singles,
            tc.tile_pool(name="xpool", bufs=3) as xpool,
            tc.tile_pool(name="xwin", bufs=2) as xwin_pool,
            tc.tile_pool(name="u_sb", bufs=4) as u_sb_pool,
            tc.tile_pool(name="xt_sb", bufs=8) as xt_pool,
            tc.tile_pool(name="z_sb", bufs=4) as z_sb_pool,
            tc.tile_pool(name="g_sb", bufs=3) as g_sb_pool,
            tc.tile_pool(name="attn_sb", bufs=2) as attn_pool,
            tc.tile_pool(name="ow_sb", bufs=6) as ow_pool,
            tc.tile_pool(name="small_sb", bufs=2) as small_pool,
            tc.tile_pool(name="ps_work", bufs=2, space="PSUM") as ps_work,
            tc.tile_pool(name="ps_dots", bufs=1, space="PSUM") as ps_dots,
            tc.tile_pool(name="ps_z", bufs=1, space="PSUM") as ps_z_pool,
            tc.tile_pool(name="ps_av", bufs=2, space="PSUM") as ps_av,
            tc.tile_pool(name="ps_o", bufs=2, space="PSUM") as ps_o_pool,
        ):
            # ---- weights / constants (resident) ----
            m_sb = singles.tile([128, IN_C], bf16)
            nc.sync.dma_start(out=m_sb, in_=m_d.ap())
            h_sb = singles.tile([128, 1], bf16)
            nc.sync.dma_start(out=h_sb, in_=h_d.ap())
            wvT = singles.tile([128, HIDE_C], bf16)
            nc.sync.dma_start(out=wvT, in_=wvT_d.ap())
            woT = singles.tile([128, 2, OUT_C], bf16)
            nc.sync.dma_start(
                out=woT, in_=woT_d.ap().rearrange("(kc p) m -> p kc m", kc=2)
            )
            biases = singles.tile([128, 3], f32)
            nc.sync.dma_start(out=biases, in_=bias_d.ap())
            bv_ap = [biases[:, 0:1], biases[:, 1:2]]
            bo_ap = biases[:, 2:3]
            ident = singles.tile([128, IN_C], bf16)
            nc.sync.dma_start(out=ident, in_=id_d.ap())

            ones_mat = singles.tile([128, 128], bf16)
            nc.vector.memset(ones_mat, 1.0)
            ones_row = singles.tile([1, T], bf16)
            nc.vector.memset(ones_row, 1.0)
            scratch = singles.tile([128, 1], f32)
            # preload the EXP ACT table once, off any critical chain
            nc.scalar.activation(scratch, biases[:, 0:1], AF.Exp)

            st = [dict() for _ in range(bpc)]
            chunk_starts = list(range(0, NP, 2))   # 2 positions per chunk

            def emit_load(img, skip_x=False):
                if not skip_x:
                    x_img = xpool.tile([128, HW], bf16, tag="ximg")
                    for dc in range(4):
                        nc.sync.dma_start(
                            out=x_img[:, dc * (HW // 4):
                                      (dc + 1) * (HW // 4)],
                            in_=x_d.ap()[img, :, dc * (HW // 4):
                                         (dc + 1) * (HW // 4)])
                    st[img]["x5"] = x_img.rearrange(
                        "p (h a w b) -> p h a w b", h=H1, a=WS, b=WS)
                st[img]["xT5"] = xT_d.ap()[img].rearrange(
                    "(h a w b) c -> h a w b c", h=H1, a=WS, b=WS)
                st[img]["xt"] = {}

            def grp_src(img, g):
                """strided image-layout AP of this group's positions"""
                x5 = st[img]["x5"]
                (ws1, ws2), p2, _ = g
                if p2 is None:
                    return x5[:, :, ws1, :, ws2]
                if p2[0] == ws1:  # within-row pair, pixel stride 1
                    return x5[:, :, ws1, :, ws2:ws2 + 2].rearrange(
                        "p h w b -> p b h w")
                return x5[:, :, ws1:ws1 + 2, :, ws2].rearrange(
                    "p h a w -> p a h w")

            def emit_winb_load(img):
                x_winb = xwin_pool.tile([128, NP * T], bf16, tag="xwin")
                st[img]["xw"] = x_winb
                for dc in range(4):
                    nc.sync.dma_start(
                        out=x_winb[:, dc * (NP * T // 4):
                                   (dc + 1) * (NP * T // 4)],
                        in_=xwin_d.ap()[img, :, dc * (NP * T // 4):
                                        (dc + 1) * (NP * T // 4)])

            def emit_permute(img, spread=True):
                """Position-major bf16 window copy, split across the three
                copy engines: GPSIMD alone is ~1.9us/copy and a serial
                25-copy chain gates the next image's load (x_img buffer
                reuse) and, through the Sync DMA sequencer, the stores."""
                x_winb = xwin_pool.tile([128, NP * T], bf16, tag="xwin")
                st[img]["xw"] = x_winb
                for gi, g in enumerate(groups):
                    N = T if g[1] is None else 2 * T
                    dst = x_winb[:, g[2]:g[2] + N]
                    src = grp_src(img, g)
                    if gi % 4 == 2:
                        nc.scalar.activation(dst, src, AF.Copy, scale=1.0)
                    elif gi % 4 == 3:
                        nc.vector.tensor_copy(dst, src)
                    else:
                        nc.gpsimd.tensor_copy(dst, src)

            # ---- phase 1: dots_T and c accumulation over positions ----
            def p1_steps(img):
                xw = st[img]["xw"]
                pend = []

                def u_conv(ci, p0):
                    npos = min(2, NP - p0)
                    N = npos * T
                    base = p0 * T
                    u_ps = ps_work.tile([128, 512], f32, tag="pwork")
                    nc.tensor.matmul(u_ps[:, :N], lhsT=m_sb,
                                     rhs=xw[:, base:base + N],
                                     start=True, stop=True)
                    u_sbt = u_sb_pool.tile([128, 512], bf16, tag="u")
                    if ci % 3 == 0:
                        nc.scalar.activation(u_sbt[:, :N], u_ps[:, :N],
                                             AF.Copy, scale=1.0)
                    else:
                        nc.vector.tensor_copy(u_sbt[:, :N], u_ps[:, :N])
                    return u_sbt

                def dots_mms(ci, p0, u_sbt):
                    dots = st[img]["dots"]
                    npos = min(2, NP - p0)
                    base = p0 * T
                    first = ci == 0
                    for pi in range(npos):
                        for jh in (0, 1):
                            nc.tensor.matmul(
                                dots[jh],
                                lhsT=u_sbt[:, pi * T + jh * 128:
                                           pi * T + jh * 128 + 128],
                                rhs=xw[:, base + pi * T:
                                       base + (pi + 1) * T],
                                start=first and pi == 0 and jh == 0,
                                stop=False,
                                skip_group_check=True)

                def mk_chunk(ci, p0):
                    def f():
                        if ci == 0:
                            dots_t = ps_dots.tile([128, 512], f32,
                                                  tag="dots", name="dots")
                            st[img]["dots"] = [dots_t[:, 0:T],
                                               dots_t[:, T:2 * T]]
                        u_sbt = u_conv(ci, p0)
                        if len(pend) >= 2:
                            dots_mms(*pend.pop(0))
                        pend.append((ci, p0, u_sbt))
                    return f

                def c_step():
                    for pe_ in pend:
                        dots_mms(*pe_)
                    pend.clear()
                    # c[j] = sum_p h . x_p[:, j] (h stays loaded), added
                    # into dots via two rank-1 matmuls
                    dots = st[img]["dots"]
                    c_row_ps = ps_o_pool.tile([1, T], f32, tag="ops",
                                              name="cps")
                    for p in range(NP):
                        nc.tensor.matmul(c_row_ps, lhsT=h_sb,
                                         rhs=xw[:, p * T:(p + 1) * T],
                                         start=p == 0, stop=p == NP - 1)
                    c_row = small_pool.tile([1, T], bf16, tag="csb")
                    nc.vector.tensor_copy(c_row, c_row_ps)
                    for jh in (0, 1):
                        nc.tensor.matmul(
                            dots[jh],
                            lhsT=c_row[:, jh * 128:jh * 128 + 128],
                            rhs=ones_row, start=False, stop=jh == 1,
                            skip_group_check=True)

                steps = [mk_chunk(ci, p0)
                         for ci, p0 in enumerate(chunk_starts)]
                steps.append(c_step)
                return steps

            # ---- token-major x^T gathers (one DMA unit covers a row of
            # up to 3 groups; contiguous (b, c) runs of 1536B) ----
            def emit_gather(img, u):
                xT5 = st[img]["xT5"]
                if u < 7:
                    t = xt_pool.tile([128, 2, 768], bf16, tag="xtrow")
                    for jh in (0, 1):
                        nc.sync.dma_start(
                            out=t[:, jh, :],
                            in_=xT5[jh * 8:jh * 8 + 8, u, :, 0:6, :])
                elif u < 10:
                    a2 = 2 * (u - 7)
                    t = xt_pool.tile([128, 2, 256], bf16, tag="xtcol")
                    for jh in (0, 1):
                        for pi in (0, 1):
                            nc.sync.dma_start(
                                out=t[:, jh, pi * 128:(pi + 1) * 128],
                                in_=xT5[jh * 8:jh * 8 + 8, a2 + pi, :, 6, :])
                else:
                    t = xt_pool.tile([128, 2, 128], bf16, tag="xtsing")
                    for jh in (0, 1):
                        nc.sync.dma_start(
                            out=t[:, jh, :],
                            in_=xT5[jh * 8:jh * 8 + 8, 6, :, 6, :])
                st[img]["xt"][u] = t

            def xt_block(img, gi, pi, jh):
                """[K=128 tokens of half jh, M=128 ch] stationary block for
                position pi of group gi."""
                if gi < 21:
                    t = st[img]["xt"][gi // 3]
                    off = ((gi % 3) * 2 + pi) * 128
                elif gi < 24:
                    t = st[img]["xt"][7 + (gi - 21)]
                    off = pi * 128
                else:
                    t = st[img]["xt"][10]
                    off = 0
                return t[:, jh, off:off + 128]

            def unit_of(gi):
                return gi // 3 if gi < 21 else 7 + min(gi - 21, 3)

            # ---- softmax over j (= partitions of dots_T) ----
            def emit_softmax(img):
                # all x^T gathers for this image up front: they depend
                # only on DRAM, and early issue keeps the 900ns DMA
                # semaphore propagation off every z-matmul's critical path
                for u in range(11):
                    emit_gather(img, u)
                dots = st[img]["dots"]
                attn = [attn_pool.tile([128, T], bf16, tag=f"attn{jc}",
                                       name=f"attn{jc}") for jc in (0, 1)]
                for jc in (0, 1):
                    nc.scalar.activation(attn[jc], dots[jc], AF.Exp)
                s_ps = ps_dots.tile([128, T], f32, tag="dots", name="ssum")
                for jc in (0, 1):
                    nc.tensor.matmul(s_ps, lhsT=ones_mat, rhs=attn[jc],
                                     start=jc == 0, stop=jc == 1)
                r_sb = small_pool.tile([128, T], f32, tag="rsb")
                # sums are positive and O(1..300): the ~18-bit approximate
                # reciprocal is plenty and ~5x faster on the critical chain
                nc.vector.reciprocal_approx_fast(r_sb, s_ps)
                # dummy gelu: pull the GELU ACT-table load off the
                # av(0) -> gelu1(0) -> out(0) critical chain
                nc.scalar.activation(scratch, biases[:, 0:1], AF.Gelu)
                for jc in (0, 1):
                    nc.vector.tensor_mul(attn[jc], attn[jc], r_sb)
                st[img]["attn"] = attn
                st[img]["zcache"] = {}

            # ---- phase 2: z = x @ attn^T, av = wv @ z, out-conv,
            # residual, store ----
            GLOOK = 12  # gather-unit prefetch distance (groups)

            def p2_steps(img):
                attn = st[img]["attn"]
                pend_out = [None]

                def emit_out(gi, g, g_tiles):
                    cnt = 1 if g[1] is None else 2
                    N = cnt * T
                    o_ps = ps_o_pool.tile([128, 512], f32, tag="ops")
                    for pi in range(cnt):
                        for kc in (0, 1):
                            nc.tensor.matmul(
                                o_ps[:, pi * T:(pi + 1) * T],
                                lhsT=woT[:, kc, :],
                                rhs=g_tiles[kc][:, pi * T:(pi + 1) * T],
                                start=kc == 0, stop=False)
                        # residual add on PE: x^T block stationary,
                        # identity moving
                        for jh in (0, 1):
                            nc.tensor.matmul(
                                o_ps[:, pi * T + jh * 128:
                                     pi * T + jh * 128 + 128],
                                lhsT=xt_block(img, gi, pi, jh),
                                rhs=ident,
                                start=False, stop=True,
                                skip_group_check=True)
                    # final gelu into a contiguous window-layout tile,
                    # stored immediately (host un-permutes)
                    ow = ow_pool.tile([128, 512], f32, tag="ow")
                    nc.scalar.activation(ow[:, :N], o_ps[:, :N], AF.Gelu,
                                         bias=bo_ap, scale=1.0)
                    nc.sync.dma_start(
                        out=out_d.ap()[img, :, g[2]:g[2] + N],
                        in_=ow[:, :N])

                zcache = st[img]["zcache"]

                def emit_z(gi, g):
                    """z_p = x_p @ attn^T: x^T blocks stationary; drained
                    to bf16 SBUF one group ahead of the wv matmuls."""
                    cnt = 1 if g[1] is None else 2
                    z_ps = ps_z_pool.tile([128, 512], f32, tag="z")
                    zsb = []
                    for pi in range(cnt):
                        for jh in (0, 1):
                            nc.tensor.matmul(
                                z_ps[:, pi * T:(pi + 1) * T],
                                lhsT=xt_block(img, gi, pi, jh),
                                rhs=attn[jh],
                                start=jh == 0, stop=jh == 1)
                        z_t = z_sb_pool.tile([128, T], bf16, tag="zsb")
                        if gi % 3 == 2:
                            nc.scalar.activation(z_t, z_ps[:, pi * T:
                                                 (pi + 1) * T],
                                                 AF.Copy, scale=1.0)
                        else:
                            nc.vector.tensor_copy(z_t, z_ps[:, pi * T:
                                                  (pi + 1) * T])
                        zsb.append(z_t)
                    zcache[gi] = zsb

                def mk_group(gi, g):
                    def f():
                        cnt = 1 if g[1] is None else 2
                        N = cnt * T
                        if gi == 0:
                            emit_z(0, groups[0])
                        if gi + 1 < len(groups):
                            emit_z(gi + 1, groups[gi + 1])
                        zsb = zcache.pop(gi)
                        g_tiles = []
                        for kc in (0, 1):
                            av = ps_av.tile([128, 512], f32, tag="av",
                                            name=f"av{kc}")
                            for pi in range(cnt):
                                nc.tensor.matmul(
                                    av[:, pi * T:(pi + 1) * T],
                                    lhsT=wvT[:, kc * 128:kc * 128 + 128],
                                    rhs=zsb[pi],
                                    start=True, stop=True)
                            g_t = g_sb_pool.tile([128, 512], bf16,
                                                 tag=f"g{kc}")
                            nc.scalar.activation(g_t[:, :N], av[:, :N],
                                                 AF.Gelu, bias=bv_ap[kc],
                                                 scale=1.0)
                            g_tiles.append(g_t)
                        # out-conv lags one group so PE never waits on gelu1
                        if pend_out[0] is not None:
                            emit_out(*pend_out[0])
                        pend_out[0] = (gi, g, g_tiles)
                    return f

                steps = [mk_group(gi, g) for gi, g in enumerate(groups)]
                steps.append(lambda: emit_out(*pend_out[0]))
                return steps

            # ---- emission schedule: splice phase 1 of image i into the
            # tail of phase 2 of image i-1 ----
            emit_load(0, skip_x=True)
            emit_winb_load(0)
            tail = []
            for img in range(bpc):
                p1 = p1_steps(img)
                # interleave previous image's phase-2 tail with this phase 1
                ti = 0
                for si, s in enumerate(p1):
                    s()
                    if (si + 1) % 3 == 0 and ti < len(tail):
                        tail[ti]()
                        ti += 1
                while ti < len(tail):
                    tail[ti]()
                    ti += 1
                if img + 1 < bpc:
                    emit_load(img + 1)
                emit_softmax(img)
                p2 = p2_steps(img)
                for s in p2[:FRONT]:
                    s()
                tail = p2[FRONT:]
                if img + 1 < bpc:
                    emit_permute(img + 1)
            for s in tail:
                s()

    nc.compile()
    return nc


def fold_params(wq, gq, bq, mq, vq, wk, gk, bk, mk, vk,
                wv, gv, bv, mv, vv, wo, bo, go, bbo, mo, vo):
    """Host-side BN/bias folding. Returns (M, h, wvT, woT, biases, ident)."""
    import ml_dtypes
    bf16 = ml_dtypes.bfloat16

    aq = gq / np.sqrt(vq + EPS)
    wq_f = (SCALE * aq)[:, None] * wq
    Bq = SCALE * (bq - aq * mq)

    ak = gk / np.sqrt(vk + EPS)
    wk_f = ak[:, None] * wk          # k bias drops (softmax shift invariance)

    M = wk_f.T @ wq_f                # dots_T = sum_p (M^T x_p)^T x_p
    hv = wk_f.T @ Bq                 # c[j] = sum_p hv . x_p[:, j]

    av = gv / np.sqrt(vv + EPS)
    wv_f = av[:, None] * wv
    Bv = bv - av * mv                # applied inside the first gelu

    ao = go / np.sqrt(vo + EPS)
    wo_f = ao[:, None] * wo
    Bo = ao * (bo - mo) + bbo        # conv bias + BN fold, inside last gelu

    biases = np.stack([Bv[:128], Bv[128:], Bo], axis=1).astype(F32)
    return (np.ascontiguousarray(M).astype(bf16),
            np.ascontiguousarray(hv[:, None]).astype(bf16),
            np.ascontiguousarray(wv_f.T).astype(bf16),
            np.ascontiguousarray(wo_f.T).astype(bf16),
            biases,
            np.eye(IN_C, dtype=bf16))


_CACHED = {}


def _get_nc(bpc=BPC):
    if bpc not in _CACHED:
        _CACHED[bpc] = build_bass_kernel(bpc)
    return _CACHED[bpc]


def make_in_maps(inputs):
    x = np.asarray(inputs["x"], F32)
    m, hv, wvT, woT, biases, ident = fold_params(
        *[np.asarray(inputs[k], F32) for k in
          ("wq", "gq", "bq", "mq", "vq", "wk", "gk", "bk", "mk", "vk",
           "wv", "gv", "bv", "mv", "vv", "wo", "bo", "go", "bbo", "mo", "vo")]
    )
    import ml_dtypes
    xb = x.astype(ml_dtypes.bfloat16)
    in_maps = []
    for c in range(NCORES):
        xs = np.ascontiguousarray(
            xb[c * BPC:(c + 1) * BPC].reshape(BPC, IN_C, HW))
        xt = np.ascontiguousarray(xs.transpose(0, 2, 1))
        xw = np.ascontiguousarray(_permute_host(
            xs.reshape(BPC, IN_C, H, W)))
        in_maps.append({"x": xs, "xT": xt, "xwin": xw, "m": m, "hcol": hv,
                        "wvT": wvT, "woT": woT, "biases": biases,
                        "ident": ident})
    return in_maps


def _blk_map():
    """Device window-layout block index for window position (a, b)."""
    blk = np.empty((WS, WS), np.int64)
    for a in range(WS):
        for b in range(WS):
            if b < 6:
                blk[a, b] = (3 * a + b // 2) * 2 + (b % 2)
            elif a < 6:
                blk[a, b] = 42 + (a // 2) * 2 + (a % 2)
            else:
                blk[a, b] = 48
    return blk


def _permute_host(x):
    """[N, C, H, W] image layout -> [N, C, NP*T] device window layout."""
    blk = _blk_map()
    order = np.empty(NP, np.int64)
    for a in range(WS):
        for b in range(WS):
            order[blk[a, b]] = a * WS + b
    t = x.reshape(x.shape[0], IN_C, H1, WS, W1, WS)
    t = t.transpose(0, 1, 3, 5, 2, 4)         # n c a b h w
    t = t.reshape(x.shape[0], IN_C, NP, T)[:, :, order]
    return np.ascontiguousarray(t.reshape(x.shape[0], IN_C, NP * T))


def _unpermute_host(res):
    """[BPC, C, NP*T] window-layout -> [BPC, C, H, W] image layout."""
    blk = _blk_map()
    t = res.reshape(res.shape[0], OUT_C, NP, H1, W1)[:, :, blk.ravel()]
    t = t.reshape(res.shape[0], OUT_C, WS, WS, H1, W1)      # c a b h w
    t = t.transpose(0, 1, 4, 2, 5, 3)                       # c h a w b
    return np.ascontiguousarray(
        t.reshape(res.shape[0], OUT_C, H, W))


def kernel(**inputs):
    from concourse.bass_utils import run_bass_kernel_spmd

    in_maps = make_in_maps(inputs)
    nc = _get_nc(BPC)
    res = run_bass_kernel_spmd(nc, in_maps, list(range(NCORES)))
    outs = [_unpermute_host(res.results[c]["out"].reshape(BPC, OUT_C, NP * T))
            for c in range(NCORES)]
    return np.concatenate(outs, axis=0)


# revision 46
# speedup vs baseline: 1.2235x; 1.0066x over previous
"""
Trainium2 Bass kernel for nn_Attention_335007449901 (sparse window attention).

Model (per image, eval mode):
  q = BN(conv1x1(x, wq)); k = BN(conv1x1(x, wk)); v = BN(conv1x1(x, wv))
  7x7 windows over the 112x112 image -> T=256 window tokens, token
  features = (channel, within-window position p) pairs.
  dots[i,j] = <q_i, k_j> * 0.125 ; attn = softmax_j ; out = attn @ v
  y = gelu(out); z = BN(conv1x1(y, wo) + bo); out = gelu(z + x)

Sharding: pure data parallel over batch, 4 images per core on 8 cores.

Implementation notes:
  * BatchNorms folded into conv weights on the host; SCALE folded into q's
    path; k's bias drops (softmax shift invariance along the normalized
    axis); v's bias passes through the attention average (rows sum to 1)
    into the first gelu's bias; the final conv bias + BN fold into the last
    gelu's bias.
  * q and k never materialize: dots_T[j,i] = sum_p x_pj^T M x_pi with
    M = wk_f^T wq_f precomputed on the host, computed as u_p = M^T x_p
    then dots_T += u_p^T x_p. q's bias contributes a per-row term
    c[j] = sum_p (wk_f^T Bq) . x_p[:,j], accumulated with rank-reduce
    matmuls and injected into dots via two rank-1 matmuls.
  * All matmul operands are bf16 (fp32 PSUM accumulation): fp32 matmuls on
    trn2 run as LOW/HIGH double passes, and strided moving operands stream
    ~5x slower, so a window-permuted contiguous bf16 copy of x (x_winb)
    feeds every matmul.
  * dots are computed transposed so softmax normalization is a ones-vector
    matmul reduce; no max subtraction needed (|dots| < ~30, fp32 exp safe).
  * The attention-average is reordered as wv @ (x @ attn^T) ("z-form"):
    the host supplies a pixel-major transposed copy of x, DMA-gathered
    into token-major SBUF blocks (contiguous 1536B runs), so z = x@attn^T
    runs with x^T as the stationary operand - no v-conv, ~25% fewer PE
    rows and half the PSUM->SBUF drain volume of the v-based form.
  * The residual add is a PE matmul accumulation (x^T block stationary,
    identity moving) - no Vector-engine tensor_tensor adds.
  * The final gelu writes CONTIGUOUS window-layout tiles that are DMA'd out
    per group as they complete (a strided in-image scatter write costs ~2x
    on the ACT engine and serializes the store into an end-of-image tail);
    the host un-permutes the window-layout result back to image layout.
  * Cross-image software pipelining: phase 1 of image i+1 (u/dots/c) is
    emitted interleaved with the tail groups of image i's phase 2 so the
    PE stream never drains at image boundaries (keeps the tensor engine's
    DVFS p-state at max).
"""

import numpy as np

IN_C = 128
HIDE_C = 256
HC2 = 128
OUT_C = 128
WS = 7
SCALE = 0.125
EPS = 1e-5
B, H, W = 32, 112, 112
HW = H * W          # 12544
H1 = H // WS        # 16
W1 = W // WS        # 16
T = H1 * W1         # 256 windows
NP = WS * WS        # 49 positions
NCORES = 8
BPC = B // NCORES   # images per core

F32 = np.float32


def _pgroups():
    """Groups of 1-2 within-window positions with a uniform pixel-offset
    stride between members (one strided access pattern per group in the
    image layout). 49 positions -> 24 pairs + 1 singleton. g[2] is the
    group's column base in the position-major window layout x_winb."""
    groups = []
    base = 0
    for ws1 in range(WS):
        for b2 in range(3):
            groups.append(((ws1, 2 * b2), (ws1, 2 * b2 + 1), base))
            base += 2 * T
    for a in range(3):
        groups.append(((2 * a, 6), (2 * a + 1, 6), base))
        base += 2 * T
    groups.append(((6, 6), None, base))
    return groups


def build_bass_kernel(bpc=BPC):
    import concourse.bass as bass
    import concourse.tile as tile
    import concourse.mybir as mybir
    from concourse import bacc

    f32 = mybir.dt.float32
    bf16 = mybir.dt.bfloat16
    AF = mybir.ActivationFunctionType

    nc = bacc.Bacc("TRN2", target_bir_lowering=False)

    # x is pre-converted to bf16 on the host: every on-device consumer
    # (window permute -> x_winb) is bf16 anyway, and this halves both the
    # load DMA volume and the x_img SBUF footprint (enabling bufs=3 so
    # loads are never gated on a recent permute's completion)
    x_d = nc.dram_tensor("x", [bpc, IN_C, HW], bf16, kind="ExternalInput")
    # pixel-major transpose of x for token-major (stationary-side) gathers
    xT_d = nc.dram_tensor("xT", [bpc, HW, IN_C], bf16, kind="ExternalInput")
    # host-permuted window-layout x; only image 0 reads it (loading it
    # directly skips the on-device permute that otherwise paces startup)
    xwin_d = nc.dram_tensor("xwin", [bpc, IN_C, NP * T], bf16,
                            kind="ExternalInput")
    m_d = nc.dram_tensor("m", [IN_C, IN_C], bf16, kind="ExternalInput")
    h_d = nc.dram_tensor("hcol", [IN_C, 1], bf16, kind="ExternalInput")
    wvT_d = nc.dram_tensor("wvT", [IN_C, HIDE_C], bf16, kind="ExternalInput")
    woT_d = nc.dram_tensor("woT", [HIDE_C, OUT_C], bf16, kind="ExternalInput")
    id_d = nc.dram_tensor("ident", [IN_C, IN_C], bf16, kind="ExternalInput")
    # packed per-partition fp32 bias columns: [Bv_lo, Bv_hi, Bo]
    bias_d = nc.dram_tensor("biases", [128, 3], f32, kind="ExternalInput")
    # output stays in window-permuted layout; the host un-permutes
    out_d = nc.dram_tensor("out", [bpc, OUT_C, NP * T], f32,
                           kind="ExternalOutput")

    groups = _pgroups()
    NEARLY = 6
    FRONT = 17   # phase-2 groups emitted before the next image's phase 1

    with tile.TileContext(nc) as tc:
        with (
            tc.tile_pool(name="singles", bufs=1) as singles,
            tc.tile_pool(name="xpool", bufs=3) as xpool,
            tc.tile_pool(name="xwin", bufs=2) as xwin_pool,
            tc.tile_pool(name="u_sb", bufs=4) as u_sb_pool,
            tc.tile_pool(name="xt_sb", bufs=8) as xt_pool,
            tc.tile_pool(name="z_sb", bufs=4) as z_sb_pool,
            tc.tile_pool(name="g_sb", bufs=3) as g_sb_pool,
            tc.tile_pool(name="attn_sb", bufs=2) as attn_pool,
            tc.tile_pool(name="ow_sb", bufs=6) as ow_pool,
            tc.tile_pool(name="small_sb", bufs=2) as small_pool,
            tc.tile_pool(name="ps_work", bufs=2, space="PSUM") as ps_work,
            tc.tile_pool(name="ps_dots", bufs=1, space="PSUM") as ps_dots,
            tc.tile_pool(name="ps_z", bufs=1, space="PSUM") as ps_z_pool,
            tc.tile_pool(name="ps_av", bufs=2, space="PSUM") as ps_av,
            tc.tile_pool(name="ps_o", bufs=2, space="PSUM") as ps_o_pool,
        ):
            # ---- weights / constants (resident) ----
            m_sb = singles.tile([128, IN_C], bf16)
            nc.sync.dma_start(out=m_sb, in_=m_d.ap())
            h_sb = singles.tile([128, 1], bf16)
            nc.sync.dma_start(out=h_sb, in_=h_d.ap())
            wvT = singles.tile([128, HIDE_C], bf16)
            nc.sync.dma_start(out=wvT, in_=wvT_d.ap())
            woT = singles.tile([128, 2, OUT_C], bf16)
            nc.sync.dma_start(
                out=woT, in_=woT_d.ap().rearrange("(kc p) m -> p kc m", kc=2)
            )
            biases = singles.tile([128, 3], f32)
            nc.sync.dma_start(out=biases, in_=bias_d.ap())
            bv_ap = [biases[:, 0:1], biases[:, 1:2]]
            bo_ap = biases[:, 2:3]
            ident = singles.tile([128, IN_C], bf16)
            nc.sync.dma_start(out=ident, in_=id_d.ap())

            ones_mat = singles.tile([128, 128], bf16)
            nc.vector.memset(ones_mat, 1.0)
            ones_row = singles.tile([1, T], bf16)
            nc.vector.memset(ones_row, 1.0)
            scratch = singles.tile([128, 1], f32)
            # preload the EXP ACT table once, off any critical chain
            nc.scalar.activation(scratch, biases[:, 0:1], AF.Exp)

            st = [dict() for _ in range(bpc)]
            chunk_starts = list(range(0, NP, 2))   # 2 positions per chunk

            def emit_load(img, skip_x=False):
                if not skip_x:
                    x_img = xpool.tile([128, HW], bf16, tag="ximg")
                    for dc in range(4):
                        nc.sync.dma_start(
                            out=x_img[:, dc * (HW // 4):
                                      (dc + 1) * (HW // 4)],
                            in_=x_d.ap()[img, :, dc * (HW // 4):
                                         (dc + 1) * (HW // 4)])
                    st[img]["x5"] = x_img.rearrange(
                        "p (h a w b) -> p h a w b", h=H1, a=WS, b=WS)
                st[img]["xT5"] = xT_d.ap()[img].rearrange(
                    "(h a w b) c -> h a w b c", h=H1, a=WS, b=WS)
                st[img]["xt"] = {}

            def grp_src(img, g):
                """strided image-layout AP of this group's positions"""
                x5 = st[img]["x5"]
                (ws1, ws2), p2, _ = g
                if p2 is None:
                    return x5[:, :, ws1, :, ws2]
                if p2[0] == ws1:  # within-row pair, pixel stride 1
                    return x5[:, :, ws1, :, ws2:ws2 + 2].rearrange(
                        "p h w b -> p b h w")
                return x5[:, :, ws1:ws1 + 2, :, ws2].rearrange(
                    "p h a w -> p a h w")

            def emit_winb_load(img):
                x_winb = xwin_pool.tile([128, NP * T], bf16, tag="xwin")
                st[img]["xw"] = x_winb
                for dc in range(4):
                    nc.sync.dma_start(
                        out=x_winb[:, dc * (NP * T // 4):
                                   (dc + 1) * (NP * T // 4)],
                        in_=xwin_d.ap()[img, :, dc * (NP * T // 4):
                                        (dc + 1) * (NP * T // 4)])

            def emit_permute(img, spread=True):
                """Position-major bf16 window copy, split across the three
                copy engines: GPSIMD alone is ~1.9us/copy and a serial
                25-copy chain gates the next image's load (x_img buffer
                reuse) and, through the Sync DMA sequencer, the stores."""
                x_winb = xwin_pool.tile([128, NP * T], bf16, tag="xwin")
                st[img]["xw"] = x_winb
                for gi, g in enumerate(groups):
                    N = T if g[1] is None else 2 * T
                    dst = x_winb[:, g[2]:g[2] + N]
                    src = grp_src(img, g)
                    if gi % 4 == 2:
                        nc.scalar.activation(dst, src, AF.Copy, scale=1.0)
                    elif gi % 4 == 3:
                        nc.vector.tensor_copy(dst, src)
                    else:
                        nc.gpsimd.tensor_copy(dst, src)

            # ---- phase 1: dots_T and c accumulation over positions ----
            def p1_steps(img):
                xw = st[img]["xw"]
                pend = []

                def u_conv(ci, p0):
                    npos = min(2, NP - p0)
                    N = npos * T
                    base = p0 * T
                    u_ps = ps_work.tile([128, 512], f32, tag="pwork")
                    nc.tensor.matmul(u_ps[:, :N], lhsT=m_sb,
                                     rhs=xw[:, base:base + N],
                                     start=True, stop=True)
                    u_sbt = u_sb_pool.tile([128, 512], bf16, tag="u")
                    if ci % 3 == 0:
                        nc.scalar.activation(u_sbt[:, :N], u_ps[:, :N],
                                             AF.Copy, scale=1.0)
                    else:
                        nc.vector.tensor_copy(u_sbt[:, :N], u_ps[:, :N])
                    return u_sbt

                def dots_mms(ci, p0, u_sbt):
                    dots = st[img]["dots"]
                    npos = min(2, NP - p0)
                    base = p0 * T
                    first = ci == 0
                    for pi in range(npos):
                        for jh in (0, 1):
                            nc.tensor.matmul(
                                dots[jh],
                                lhsT=u_sbt[:, pi * T + jh * 128:
                                           pi * T + jh * 128 + 128],
                                rhs=xw[:, base + pi * T:
                                       base + (pi + 1) * T],
                                start=first and pi == 0 and jh == 0,
                                stop=False,
                                skip_group_check=True)

                def mk_chunk(ci, p0):
                    def f():
                        if ci == 0:
                            dots_t = ps_dots.tile([128, 512], f32,
                                                  tag="dots", name="dots")
                            st[img]["dots"] = [dots_t[:, 0:T],
                                               dots_t[:, T:2 * T]]
                        u_sbt = u_conv(ci, p0)
                        if len(pend) >= 2:
                            dots_mms(*pend.pop(0))
                        pend.append((ci, p0, u_sbt))
                    return f

                def c_step():
                    for pe_ in pend:
                        dots_mms(*pe_)
                    pend.clear()
                    # c[j] = sum_p h . x_p[:, j] (h stays loaded), added
                    # into dots via two rank-1 matmuls
                    dots = st[img]["dots"]
                    c_row_ps = ps_o_pool.tile([1, T], f32, tag="ops",
                                              name="cps")
                    for p in range(NP):
                        nc.tensor.matmul(c_row_ps, lhsT=h_sb,
                                         rhs=xw[:, p * T:(p + 1) * T],
                                         start=p == 0, stop=p == NP - 1)
                    c_row = small_pool.tile([1, T], bf16, tag="csb")
                    nc.vector.tensor_copy(c_row, c_row_ps)
                    for jh in (0, 1):
                        nc.tensor.matmul(
                            dots[jh],
                            lhsT=c_row[:, jh * 128:jh * 128 + 128],
                            rhs=ones_row, start=False, stop=jh == 1,
                            skip_group_check=True)

                steps = [mk_chunk(ci, p0)
                         for ci, p0 in enumerate(chunk_starts)]
                steps.append(c_step)
                return steps

            # ---- token-major x^T gathers (one DMA unit covers a row of
            # up to 3 groups; contiguous (b, c) runs of 1536B) ----
            def emit_gather(img, u):
                xT5 = st[img]["xT5"]
                if u < 7:
                    t = xt_pool.tile([128, 2, 768], bf16, tag="xtrow")
                    for jh in (0, 1):
                        nc.sync.dma_start(
                            out=t[:, jh, :],
                            in_=xT5[jh * 8:jh * 8 + 8, u, :, 0:6, :])
                elif u < 10:
                    a2 = 2 * (u - 7)
                    t = xt_pool.tile([128, 2, 256], bf16, tag="xtcol")
                    for jh in (0, 1):
                        for pi in (0, 1):
                            nc.sync.dma_start(
                                out=t[:, jh, pi * 128:(pi + 1) * 128],
                                in_=xT5[jh * 8:jh * 8 + 8, a2 + pi, :, 6, :])
                else:
                    t = xt_pool.tile([128, 2, 128], bf16, tag="xtsing")
                    for jh in (0, 1):
                        nc.sync.dma_start(
                            out=t[:, jh, :],
                            in_=xT5[jh * 8:jh * 8 + 8, 6, :, 6, :])
                st[img]["xt"][u] = t

            def xt_block(img, gi, pi, jh):
                """[K=128 tokens of half jh, M=128 ch] stationary block for
                position pi of group gi."""
                if gi < 21:
                    t = st[img]["xt"][gi // 3]
                    off = ((gi % 3) * 2 + pi) * 128
                elif gi < 24:
                    t = st[img]["xt"][7 + (gi - 21)]
                    off = pi * 128
                else:
                    t = st[img]["xt"][10]
                    off = 0
                return t[:, jh, off:off + 128]

            def unit_of(gi):
                return gi // 3 if gi < 21 else 7 + min(gi - 21, 3)

            # ---- softmax over j (= partitions of dots_T) ----
            def emit_softmax(img):
                # all x^T gathers for this image up front: they depend
                # only on DRAM, and early issue keeps the 900ns DMA
                # semaphore propagation off every z-matmul's critical path
                for u in range(11):
                    emit_gather(img, u)
                dots = st[img]["dots"]
                attn = [attn_pool.tile([128, T], bf16, tag=f"attn{jc}",
                                       name=f"attn{jc}") for jc in (0, 1)]
                for jc in (0, 1):
                    nc.scalar.activation(attn[jc], dots[jc], AF.Exp)
                s_ps = ps_dots.tile([128, T], f32, tag="dots", name="ssum")
                for jc in (0, 1):
                    nc.tensor.matmul(s_ps, lhsT=ones_mat, rhs=attn[jc],
                                     start=jc == 0, stop=jc == 1)
                r_sb = small_pool.tile([128, T], f32, tag="rsb")
                # sums are positive and O(1..300): the ~18-bit approximate
                # reciprocal is plenty and ~5x faster on the critical chain
                nc.vector.reciprocal_approx_fast(r_sb, s_ps)
                # dummy gelu: pull the GELU ACT-table load off the
                # av(0) -> gelu1(0) -> out(0) critical chain
                nc.scalar.activation(scratch, biases[:, 0:1], AF.Gelu)
                for jc in (0, 1):
                    nc.vector.tensor_mul(attn[jc], attn[jc], r_sb)
                st[img]["attn"] = attn
                st[img]["zcache"] = {}

            # ---- phase 2: z = x @ attn^T, av = wv @ z, out-conv,
            # residual, store ----
            GLOOK = 12  # gather-unit prefetch distance (groups)

            def p2_steps(img):
                attn = st[img]["attn"]
                pend_out = [None]

                def emit_out(gi, g, g_tiles):
                    cnt = 1 if g[1] is None else 2
                    N = cnt * T
                    o_ps = ps_o_pool.tile([128, 512], f32, tag="ops")
                    for pi in range(cnt):
                        for kc in (0, 1):
                            nc.tensor.matmul(
                                o_ps[:, pi * T:(pi + 1) * T],
                                lhsT=woT[:, kc, :],
                                rhs=g_tiles[kc][:, pi * T:(pi + 1) * T],
                                start=kc == 0, stop=False)
                        # residual add on PE: x^T block stationary,
                        # identity moving
                        for jh in (0, 1):
                            nc.tensor.matmul(
                                o_ps[:, pi * T + jh * 128:
                                     pi * T + jh * 128 + 128],
                                lhsT=xt_block(img, gi, pi, jh),
                                rhs=ident,
                                start=False, stop=True,
                                skip_group_check=True)
                    # final gelu into a contiguous window-layout tile,
                    # stored immediately (host un-permutes)
                    ow = ow_pool.tile([128, 512], f32, tag="ow")
                    nc.scalar.activation(ow[:, :N], o_ps[:, :N], AF.Gelu,
                                         bias=bo_ap, scale=1.0)
                    nc.sync.dma_start(
                        out=out_d.ap()[img, :, g[2]:g[2] + N],
                        in_=ow[:, :N])

                zcache = st[img]["zcache"]

                def emit_z(gi, g):
                    """z_p = x_p @ attn^T: x^T blocks stationary; drained
                    to bf16 SBUF one group ahead of the wv matmuls."""
                    cnt = 1 if g[1] is None else 2
                    z_ps = ps_z_pool.tile([128, 512], f32, tag="z")
                    zsb = []
                    for pi in range(cnt):
                        for jh in (0, 1):
                            nc.tensor.matmul(
                                z_ps[:, pi * T:(pi + 1) * T],
                                lhsT=xt_block(img, gi, pi, jh),
                                rhs=attn[jh],
                                start=jh == 0, stop=jh == 1)
                        z_t = z_sb_pool.tile([128, T], bf16, tag="zsb")
                        nc.vector.tensor_copy(z_t, z_ps[:, pi * T:
                                              (pi + 1) * T])
                        zsb.append(z_t)
                    zcache[gi] = zsb

                def mk_group(gi, g):
                    def f():
                        cnt = 1 if g[1] is None else 2
                        N = cnt * T
                        if gi == 0:
                            emit_z(0, groups[0])
                        if gi + 1 < len(groups):
                            emit_z(gi + 1, groups[gi + 1])
                        zsb = zcache.pop(gi)
                        g_tiles = []
                        for kc in (0, 1):
                            av = ps_av.tile([128, 512], f32, tag="av",
                                            name=f"av{kc}")
                            for pi in range(cnt):
                                nc.tensor.matmul(
                                    av[:, pi * T:(pi + 1) * T],
                                    lhsT=wvT[:, kc * 128:kc * 128 + 128],
                                    rhs=zsb[pi],
                                    start=True, stop=True)
                            g_t = g_sb_pool.tile([128, 512], bf16,
                                                 tag=f"g{kc}")
                            nc.scalar.activation(g_t[:, :N], av[:, :N],
                                                 AF.Gelu, bias=bv_ap[kc],
                                                 scale=1.0)
                            g_tiles.append(g_t)
                        # out-conv lags one group so PE never waits on gelu1
                        if pend_out[0] is not None:
                            emit_out(*pend_out[0])
                        pend_out[0] = (gi, g, g_tiles)
                    return f

                steps = [mk_group(gi, g) for gi, g in enumerate(groups)]
                steps.append(lambda: emit_out(*pend_out[0]))
                return steps

            # ---- emission schedule: splice phase 1 of image i into the
            # tail of phase 2 of image i-1 ----
            emit_load(0, skip_x=True)
            emit_winb_load(0)
            tail = []
            for img in range(bpc):
                p1 = p1_steps(img)
                # interleave previous image's phase-2 tail with this phase 1
                ti = 0
                for si, s in enumerate(p1):
                    s()
                    if (si + 1) % 3 == 0 and ti < len(tail):
                        tail[ti]()
                        ti += 1
                while ti < len(tail):
                    tail[ti]()
                    ti += 1
                if img + 1 < bpc:
                    emit_load(img + 1)
                emit_softmax(img)
                p2 = p2_steps(img)
                for s in p2[:FRONT]:
                    s()
                tail = p2[FRONT:]
                if img + 1 < bpc:
                    emit_permute(img + 1)
            for s in tail:
                s()

    nc.compile()
    return nc


def fold_params(wq, gq, bq, mq, vq, wk, gk, bk, mk, vk,
                wv, gv, bv, mv, vv, wo, bo, go, bbo, mo, vo):
    """Host-side BN/bias folding. Returns (M, h, wvT, woT, biases, ident)."""
    import ml_dtypes
    bf16 = ml_dtypes.bfloat16

    aq = gq / np.sqrt(vq + EPS)
    wq_f = (SCALE * aq)[:, None] * wq
    Bq = SCALE * (bq - aq * mq)

    ak = gk / np.sqrt(vk + EPS)
    wk_f = ak[:, None] * wk          # k bias drops (softmax shift invariance)

    M = wk_f.T @ wq_f                # dots_T = sum_p (M^T x_p)^T x_p
    hv = wk_f.T @ Bq                 # c[j] = sum_p hv . x_p[:, j]

    av = gv / np.sqrt(vv + EPS)
    wv_f = av[:, None] * wv
    Bv = bv - av * mv                # applied inside the first gelu

    ao = go / np.sqrt(vo + EPS)
    wo_f = ao[:, None] * wo
    Bo = ao * (bo - mo) + bbo        # conv bias + BN fold, inside last gelu

    biases = np.stack([Bv[:128], Bv[128:], Bo], axis=1).astype(F32)
    return (np.ascontiguousarray(M).astype(bf16),
            np.ascontiguousarray(hv[:, None]).astype(bf16),
            np.ascontiguousarray(wv_f.T).astype(bf16),
            np.ascontiguousarray(wo_f.T).astype(bf16),
            biases,
            np.eye(IN_C, dtype=bf16))


_CACHED = {}


def _get_nc(bpc=BPC):
    if bpc not in _CACHED:
        _CACHED[bpc] = build_bass_kernel(bpc)
    return _CACHED[bpc]


def make_in_maps(inputs):
    x = np.asarray(inputs["x"], F32)
    m, hv, wvT, woT, biases, ident = fold_params(
        *[np.asarray(inputs[k], F32) for k in
          ("wq", "gq", "bq", "mq", "vq", "wk", "gk", "bk", "mk", "vk",
           "wv", "gv", "bv", "mv", "vv", "wo", "bo", "go", "bbo", "mo", "vo")]
    )
    import ml_dtypes
    xb = x.astype(ml_dtypes.bfloat16)
    in_maps = []
    for c in range(NCORES):
        xs = np.ascontiguousarray(
            xb[c * BPC:(c + 1) * BPC].reshape(BPC, IN_C, HW))
        xt = np.ascontiguousarray(xs.transpose(0, 2, 1))
        xw = np.ascontiguousarray(_permute_host(
            xs.reshape(BPC, IN_C, H, W)))
        in_maps.append({"x": xs, "xT": xt, "xwin": xw, "m": m, "hcol": hv,
                        "wvT": wvT, "woT": woT, "biases": biases,
                        "ident": ident})
    return in_maps


def _blk_map():
    """Device window-layout block index for window position (a, b)."""
    blk = np.empty((WS, WS), np.int64)
    for a in range(WS):
        for b in range(WS):
            if b < 6:
                blk[a, b] = (3 * a + b // 2) * 2 + (b % 2)
            elif a < 6:
                blk[a, b] = 42 + (a // 2) * 2 + (a % 2)
            else:
                blk[a, b] = 48
    return blk


def _permute_host(x):
    """[N, C, H, W] image layout -> [N, C, NP*T] device window layout."""
    blk = _blk_map()
    order = np.empty(NP, np.int64)
    for a in range(WS):
        for b in range(WS):
            order[blk[a, b]] = a * WS + b
    t = x.reshape(x.shape[0], IN_C, H1, WS, W1, WS)
    t = t.transpose(0, 1, 3, 5, 2, 4)         # n c a b h w
    t = t.reshape(x.shape[0], IN_C, NP, T)[:, :, order]
    return np.ascontiguousarray(t.reshape(x.shape[0], IN_C, NP * T))


def _unpermute_host(res):
    """[BPC, C, NP*T] window-layout -> [BPC, C, H, W] image layout."""
    blk = _blk_map()
    t = res.reshape(res.shape[0], OUT_C, NP, H1, W1)[:, :, blk.ravel()]
    t = t.reshape(res.shape[0], OUT_C, WS, WS, H1, W1)      # c a b h w
    t = t.transpose(0, 1, 4, 2, 5, 3)                       # c h a w b
    return np.ascontiguousarray(
        t.reshape(res.shape[0], OUT_C, H, W))


def kernel(**inputs):
    from concourse.bass_utils import run_bass_kernel_spmd

    in_maps = make_in_maps(inputs)
    nc = _get_nc(BPC)
    res = run_bass_kernel_spmd(nc, in_maps, list(range(NCORES)))
    outs = [_unpermute_host(res.results[c]["out"].reshape(BPC, OUT_C, NP * T))
            for c in range(NCORES)]
    return np.concatenate(outs, axis=0)
